# revision 18
# baseline (speedup 1.0000x reference)
"""Trainium2 Bass kernel for nn_DGraFormer_framework (gnn_message_passing).

Reference computation (B=32, N=64, S=336, D=32, K=3 layers, beta=0.05):
    per (b, s):  A = adj[b,s]  (row-normalized [N,N])
    H0 = x w_start + b_start          [N, D]
    H_{k+1} = beta*x + (1-beta) A^T H_k
    out = concat(H_0..H_3) @ w_mlp + b_mlp   -> [b, n, s]

Everything is linear in the feature dim, so D collapses:
    out[b,:,s] = pre0 + A'(pre1 + A'(pre2 + A' pre3))      (Horner)
where A' = A^T and pre_j[b,n,s] = c_j * x[b,n,s] + d_j (scalars c_j, d_j, e
derived from w_start/b_start/w_mlp/b_mlp on the host; e folded into pre0).

Quantization: adj is stored fp8 (e3m4) scaled by 16. pre_j planes are
pre-scaled by 16^(3-j) on the host and the final output is divided by 16^3
after the gather.

Device kernel (per core; data-parallel over batch, 4 b per core):
  - adj[b] packed as 84 "quads": 4 A-matrices per 128x128 stationary tile
    (2x2 blocks of 64x64), fp8 e3m4.  Quad block (pb,cb) holds
    A_{s=4q+sigma(pb,cb)}, sigma = [[1,0],[2,3]][pb][cb]; moving col 4q+j
    carries chain s=4q+j in its input half (j0,j1 top; j2,j3 bottom),
    output lands at the class's output half (j1 top/top, j3 bot/bot direct;
    j0 top->bot, j2 bot->top crossed).
  - 3 passes of one matmul per quad. Pass transitions:
      * direct classes (j1, j3): one DVE add psum+pre -> V  per half-class.
      * crossed classes (j0, j2): DVE stages psum+pre pair-interleaved into
        TA, a PE matmul with the 64<->64 partition-swap matrix moves the
        halves, two Scalar copies land them in V.
    During the DMA-delivery-bound early steps (and for the tail batch) the
    shift+copies run inline right after their stage (PE has idle slots);
    in steady state they run at the START of the next step so their inputs
    are a full step old and the PE FIFO never blocks on them.
  - All tiles are per-batch (no parity rotation): adj, host-shipped pre
    planes (pre0..pre2 mirrored + v3 start vectors, [128, 4S] fp16 per
    batch), V1/V2 chain tiles, TA staging, TF output staging.
  - All data DMAs ride the two hardware DGE queues (SP: adj + sh16 +
    outputs; Activation: pre/v3 planes).  gpsimd only does V memsets.
  - PSUM: banks 0-6 = 7 rotating pass-psum slots; bank 7 = 3 rotating
    shift-output slots [128, 168].
  - Final pass lands psum+pre0 in TF[b] (crossed pair block / direct pair
    block); host reassembles column order (free) after the gather.
"""

import sys

sys.path.insert(0, "/opt/trn_rl_repo")

import ml_dtypes
import numpy as np

import concourse.bass as bass
import concourse.mybir as mybir
import concourse.tile as tile
from concourse import bacc
from concourse.bass_utils import run_bass_kernel_spmd

B, N, S, D = 32, 64, 336, 32
MP_LAYERS = 3
PROPBETA = 0.05
NCORES = 8
BL = B // NCORES          # batches per core
Q = S // 4                # quads per batch (84)
H = Q // 2                # half split (42 quads)

ADJ_DT = mybir.dt.float8e3    # e3m4
ADJ_NP = ml_dtypes.float8_e3m4
ADJ_SCALE = 16.0
OUT_DESCALE = float(ADJ_SCALE ** MP_LAYERS)
V_DT = mybir.dt.float16       # chain-vector / pre / out dtype
V_NP = np.float16

f32 = mybir.dt.float32


def _coefficients(w_start, b_start, w_mlp, b_mlp):
    """Collapse the feature dim: out = sum_j A'^j (c_j x + d_j 1) + e (j=0..K)."""
    K = MP_LAYERS
    beta, sb = PROPBETA, 1.0 - PROPBETA
    ws = w_start[0].astype(np.float64)
    bs = b_start.astype(np.float64)
    w = [w_mlp[k * D:(k + 1) * D, 0].astype(np.float64) for k in range(K + 1)]

    u = {(0, 0): ws}
    v = {(0, 0): bs}
    for k in range(K):
        nu = {(k + 1, 0): beta * np.ones(D)}
        nv = {(k + 1, 0): np.zeros(D)}
        for j in range(k + 1):
            nu[(k + 1, j + 1)] = sb * u[(k, j)]
            nv[(k + 1, j + 1)] = sb * v[(k, j)]
        u.update(nu)
        v.update(nv)

    c = np.zeros(K + 1)
    d = np.zeros(K + 1)
    for k in range(K + 1):
        for j in range(k + 1):
            c[j] += float(u[(k, j)] @ w[k])
            d[j] += float(v[(k, j)] @ w[k])
    e = d[0] + float(b_mlp[0])
    return c, d, e


def _shift_matrix():
    sh = np.zeros((128, 128), dtype=np.float32)
    idx = np.arange(64)
    sh[idx, idx + 64] = 1.0
    sh[idx + 64, idx] = 1.0
    return sh


def _qview(ap):
    """[P, S] -> [P, q, f] with f in 0..3 (col = 4q+f)."""
    return ap.rearrange("p (q f) -> p q f", f=4)


def build_nc():
    nc = bacc.Bacc("TRN2", target_bir_lowering=False, debug=False)

    adj_l = nc.dram_tensor("adj", [BL, 128, Q * 128], ADJ_DT,
                           kind="ExternalInput")
    pre_l = nc.dram_tensor("prev3", [BL, 128, 4 * S], V_DT,
                           kind="ExternalInput")
    shift16 = nc.dram_tensor("shift16", [128, 128], V_DT, kind="ExternalInput")
    out_l = nc.dram_tensor("out", [BL, 128, S], V_DT, kind="ExternalOutput")

    with tile.TileContext(nc) as tc:
        with (
            tc.tile_pool(name="singles", bufs=1) as singles,
            tc.tile_pool(name="psb_pool", bufs=1, space=bass.MemorySpace.PSUM)
            as psb_pool,
        ):
            sh16 = singles.tile([128, 128], V_DT, tag="sh16", name="sh16")
            pre_t = [singles.tile([128, 4 * S], V_DT, tag=f"pre{b}",
                                  name=f"pre{b}") for b in range(BL)]
            V = {}
            for b in range(BL):
                for k in (1, 2):
                    V[b, k] = singles.tile([128, S], V_DT,
                                           tag=f"v{b}{k}", name=f"v{b}{k}")
            TA = [singles.tile([128, 2 * Q], V_DT, tag=f"ta{b}", name=f"ta{b}")
                  for b in range(BL)]
            TF = [singles.tile([128, S], V_DT, tag=f"tf{b}", name=f"tf{b}")
                  for b in range(BL)]
            adj_t = [singles.tile([128, Q * 128], ADJ_DT,
                                  tag=f"adj{b}", name=f"adj{b}")
                     for b in range(BL)]

            # ---- all input DMAs up front, hardware DGE queues only ----
            for q0, q1 in ((0, 12), (12, 36), (36, 84)):
                nc.sync.dma_start(out=adj_t[0][:, q0 * 128:q1 * 128],
                                  in_=adj_l[0][:, q0 * 128:q1 * 128])
            nc.sync.dma_start(out=adj_t[1][:, :H * 128],
                              in_=adj_l[1][:, :H * 128])
            nc.sync.dma_start(sh16[:], shift16[:])
            nc.sync.dma_start(out=adj_t[1][:, H * 128:],
                              in_=adj_l[1][:, H * 128:])
            for b in (2, 3):
                nc.sync.dma_start(out=adj_t[b][:, :H * 128],
                                  in_=adj_l[b][:, :H * 128])
                nc.sync.dma_start(out=adj_t[b][:, H * 128:],
                                  in_=adj_l[b][:, H * 128:])

            nc.scalar.dma_start(out=pre_t[0][:, 3 * S:],
                                in_=pre_l[0][:, 3 * S:])
            nc.scalar.dma_start(out=pre_t[0][:, :3 * S],
                                in_=pre_l[0][:, :3 * S])
            for b in (1, 2, 3):
                nc.scalar.dma_start(out=pre_t[b][:], in_=pre_l[b][:])

            for b in range(BL):
                for k in (1, 2):
                    nc.gpsimd.memset(V[b, k][:], 0.0)

            def vsrc(b, k):
                if k == 3:
                    return pre_t[b][:, 3 * S:]
                return V[b, k][:]

            def pre_plane(b, j):
                return pre_t[b][:, j * S:(j + 1) * S]

            psb = [psb_pool.tile([128, 512], f32, tag=f"psb{i}",
                                 name=f"psb{i}") for i in range(8)]
            psums = {}
            shps = {}
            _ctr = {"ps": 0, "sh": 0}

            def ps_of(b, k):
                key = (b, k)
                if key not in psums:
                    s = _ctr["ps"] % 7
                    _ctr["ps"] += 1
                    psums[key] = psb[s][:, 0:336]
                return psums[key]

            def sh_of(b, k):
                key = (b, k)
                if key not in shps:
                    s = _ctr["sh"] % 3
                    _ctr["sh"] += 1
                    shps[key] = psb[7][:, s * 168:(s + 1) * 168]
                return shps[key]

            def p_slice(b, k, q0, q1):
                ps = ps_of(b, k)
                src = vsrc(b, k)
                for q in range(q0, q1):
                    nc.tensor.matmul(
                        ps[:, 4 * q:4 * q + 4],
                        adj_t[b][:, 128 * q:128 * (q + 1)],
                        src[:, 4 * q:4 * q + 4],
                        start=True, stop=True,
                    )

            def stage_a(b, k, q0, q1):
                p = _qview(ps_of(b, k))
                pr = _qview(pre_plane(b, k - 1))
                qs = slice(q0, q1)
                blk = TA[b][:, 2 * q0:2 * q1].rearrange("p (q c) -> p q c",
                                                        c=2)
                nc.vector.tensor_add(blk[:, :, :], p[:, qs, 0:3:2],
                                     pr[:, qs, 0:3:2])

            def b_direct(b, k, q0, q1):
                p = _qview(ps_of(b, k))
                pr = _qview(pre_plane(b, k - 1))
                vn = _qview(V[b, k - 1][:])
                qs = slice(q0, q1)
                nc.vector.tensor_add(vn[0:64, qs, 1], p[0:64, qs, 1],
                                     pr[0:64, qs, 1])
                nc.vector.tensor_add(vn[64:128, qs, 3], p[64:128, qs, 3],
                                     pr[64:128, qs, 3])

            def shift(b, k, q0, q1):
                sv = sh_of(b, k)
                nc.tensor.matmul(sv[:, 2 * q0:2 * q1], sh16[:],
                                 TA[b][:, 2 * q0:2 * q1],
                                 start=True, stop=True,
                                 skip_group_check=True)

            def copies(b, k, q0, q1):
                vn = _qview(V[b, k - 1][:])
                sv = sh_of(b, k).rearrange("p (q t) -> p q t", t=2)
                qs = slice(q0, q1)
                nc.scalar.copy(vn[0:64, qs, 0], sv[0:64, qs, 0])
                nc.scalar.copy(vn[64:128, qs, 2], sv[64:128, qs, 1])

            def f_stage(b, q0, q1):
                p1 = _qview(ps_of(b, 1))
                pr = _qview(pre_plane(b, 0))
                qs = slice(q0, q1)
                blka = TF[b][:, 2 * q0:2 * q1].rearrange("p (q c) -> p q c",
                                                         c=2)
                blkb = TF[b][:, 168 + 2 * q0:168 + 2 * q1].rearrange(
                    "p (q c) -> p q c", c=2)
                nc.vector.tensor_add(blka[:, :, :], p1[:, qs, 0:4:3],
                                     pr[:, qs, 0:4:3])
                nc.vector.tensor_add(blkb[:, :, :], p1[:, qs, 1:3],
                                     pr[:, qs, 1:3])

            def trans_inline(b, k, q0, q1):
                stage_a(b, k, q0, q1)
                b_direct(b, k, q0, q1)
                shift(b, k, q0, q1)
                copies(b, k, q0, q1)

            def trans_stage(b, k, q0, q1):
                stage_a(b, k, q0, q1)
                b_direct(b, k, q0, q1)

            # ---- 3-deep skewed software pipeline, half-granular -----------
            # Crossed-class transitions straddle step boundaries: pass-k
            # psums are staged (DVE) at the end of the step that produced
            # them, the shift matmuls + copies run AT THE START of the next
            # step (inputs a full step old -> no PE stall), before the
            # dependent pass-(k-1) matmuls of the same batch run mid-step.
            # Exception: the tail batch's k2 transition is inline at the
            # end of s4 (nothing else to overlap with in s5 anyway).
            for s in range(BL + 2):
                c = s if s < BL else None                  # pass-3 batch
                b = s - 1 if 0 <= s - 1 < BL else None     # pass-2 batch
                a = s - 2 if 0 <= s - 2 < BL else None     # pass-1 batch

                if a is not None and a < BL - 1:
                    shift(a, 2, 0, H)
                    copies(a, 2, 0, H)
                if b is not None:
                    shift(b, 3, 0, H)
                    copies(b, 3, 0, H)
                if a is not None and a < BL - 1:
                    shift(a, 2, H, Q)
                    copies(a, 2, H, Q)
                if b is not None:
                    shift(b, 3, H, Q)
                    copies(b, 3, H, Q)

                if a is not None:
                    p_slice(a, 1, 0, H)
                    f_stage(a, 0, H)
                if b is not None:
                    p_slice(b, 2, 0, H)
                    trans_stage(b, 2, 0, H)
                    if b == BL - 1:
                        shift(b, 2, 0, H)
                        copies(b, 2, 0, H)
                if c is not None and s >= 3:
                    # adj stream has caught up by now: the scheduler hoists
                    # these matmuls early, so their stages must sit early in
                    # the DVE FIFO too (production order), or next step's
                    # shifts stall behind a not-yet-ready stage
                    p_slice(c, 3, 0, H)
                    trans_stage(c, 3, 0, H)
                if a is not None:
                    p_slice(a, 1, H, Q)
                    f_stage(a, H, Q)
                    nc.sync.dma_start(out=out_l[a], in_=TF[a][:])
                if b is not None:
                    p_slice(b, 2, H, Q)
                    trans_stage(b, 2, H, Q)
                    if b == BL - 1:
                        shift(b, 2, H, Q)
                        copies(b, 2, H, Q)
                if c is not None:
                    if s < 3:
                        if c == 0:
                            # paced to the adj chunk arrivals (12, 24, 48)
                            p_slice(0, 3, 0, 12)
                            p_slice(0, 3, 12, 36)
                            p_slice(0, 3, 36, H)
                        else:
                            p_slice(c, 3, 0, H)
                        trans_stage(c, 3, 0, H)
                    p_slice(c, 3, H, Q)
                    trans_stage(c, 3, H, Q)

    nc.finalize()
    return nc


_NC_CACHE = None


def _get_nc():
    global _NC_CACHE
    if _NC_CACHE is None:
        _NC_CACHE = build_nc()
    return _NC_CACHE


def _pack_adj(adj):
    """[B, S, N, N] f32 -> [B, 128, Q*128] fp8 (x16) quad layout."""
    sigma = np.array([[1, 0], [2, 3]])  # [pb][cb]
    s_idx = 4 * np.arange(Q)[:, None, None] + sigma[None, :, :]
    a = adj[:, s_idx]                      # [B, Q, 2pb, 2cb, n, m]
    a = a.transpose(0, 2, 4, 1, 3, 5)      # [B, pb, n, Q, cb, m]
    return np.ascontiguousarray(
        (a.reshape(B, 128, Q * 128) * ADJ_SCALE).astype(ADJ_NP))


def _prepare_in_maps(x, adj, w_start, b_start, w_mlp, b_mlp):
    c, d, e = _coefficients(np.asarray(w_start), np.asarray(b_start),
                            np.asarray(w_mlp), np.asarray(b_mlp))
    x = np.asarray(x, dtype=np.float32)
    adj = _pack_adj(np.asarray(adj, dtype=np.float32))
    prev3 = np.zeros((B, 128, 4 * S), dtype=V_NP)
    for j in range(MP_LAYERS):
        sc = ADJ_SCALE ** (MP_LAYERS - j)
        plane = (c[j] * sc * x + (e if j == 0 else d[j]) * sc).astype(V_NP)
        prev3[:, 0:64, j * S:(j + 1) * S] = plane
        prev3[:, 64:128, j * S:(j + 1) * S] = plane
    pre3 = (c[MP_LAYERS] * x + d[MP_LAYERS]).astype(V_NP)  # [B, N, S]
    p3q = pre3.reshape(B, N, Q, 4)
    v3 = np.zeros((B, 128, Q, 4), dtype=V_NP)
    v3[:, 0:64, :, 0:2] = p3q[:, :, :, 0:2]
    v3[:, 64:128, :, 2:4] = p3q[:, :, :, 2:4]
    prev3[:, :, 3 * S:] = v3.reshape(B, 128, S)
    sh = _shift_matrix().astype(V_NP)
    in_maps = []
    for i in range(NCORES):
        sl = slice(i * BL, (i + 1) * BL)
        in_maps.append({
            "adj": np.ascontiguousarray(adj[sl]),
            "prev3": np.ascontiguousarray(prev3[sl]),
            "shift16": sh,
        })
    return in_maps


def run_spmd(inputs, trace=False, **kw):
    in_maps = _prepare_in_maps(**inputs)
    res = run_bass_kernel_spmd(_get_nc(), in_maps,
                               core_ids=list(range(NCORES)), trace=trace, **kw)
    tfd = np.concatenate([r["out"] for r in res.results],
                         axis=0).astype(np.float32)
    # staged blocks: A = (f0, f3) pair-interleaved, useful at partitions
    # 64:128; B = (f1, f2), useful at 0:64 (garbage halves ignored)
    a = tfd[:, 64:128, 0:2 * Q].reshape(B, N, Q, 2)
    bb = tfd[:, 0:64, 2 * Q:4 * Q].reshape(B, N, Q, 2)
    out = np.empty((B, N, Q, 4), dtype=np.float32)
    out[..., 0] = a[..., 0]
    out[..., 3] = a[..., 1]
    out[..., 1] = bb[..., 0]
    out[..., 2] = bb[..., 1]
    return out.reshape(B, N, S) / OUT_DESCALE, res


def kernel(**inputs):
    out, _ = run_spmd(inputs)
    return out.astype(np.float32)


if __name__ == "__main__":
    # quick smoke test against a numpy oracle
    rng = np.random.default_rng(0)
    x = rng.standard_normal((B, N, S), dtype=np.float32)
    adj = rng.random((B, S, N, N), dtype=np.float32)
    adj /= adj.sum(-1, keepdims=True)
    w_start = rng.standard_normal((1, D)).astype(np.float32)
    b_start = (rng.standard_normal(D) * 0.01).astype(np.float32)
    w_mlp = (rng.standard_normal(((MP_LAYERS + 1) * D, 1)) /
             np.sqrt((MP_LAYERS + 1) * D)).astype(np.float32)
    b_mlp = (rng.standard_normal(1) * 0.01).astype(np.float32)

    got = kernel(x=x, adj=adj, w_start=w_start, b_start=b_start,
                 w_mlp=w_mlp, b_mlp=b_mlp)

    h = x[..., None] * w_start[0] + b_start
    outs = [h]
    a = np.transpose(adj, (0, 2, 3, 1))
    for _ in range(MP_LAYERS):
        conv = np.einsum('bnsc,bnms->bmsc', h, a, optimize=True)
        h = PROPBETA * x[..., None] + (1 - PROPBETA) * conv
        outs.append(h)
    hc = np.concatenate(outs, axis=-1)
    want = (hc @ w_mlp)[..., 0] + b_mlp[0]

    aerr = np.abs(got - want)
    print("max abs err:", aerr.max(),
          "normalized:", aerr.max() / np.abs(want).max())


# revision 19
# speedup vs baseline: 1.0641x; 1.0641x over previous
"""Trainium2 Bass kernel for nn_DGraFormer_framework (gnn_message_passing).

Reference computation (B=32, N=64, S=336, D=32, K=3 layers, beta=0.05):
    per (b, s):  A = adj[b,s]  (row-normalized [N,N])
    H0 = x w_start + b_start          [N, D]
    H_{k+1} = beta*x + (1-beta) A^T H_k
    out = concat(H_0..H_3) @ w_mlp + b_mlp   -> [b, n, s]

Everything is linear in the feature dim, so D collapses:
    out[b,:,s] = pre0 + A'(pre1 + A'(pre2 + A' pre3))      (Horner)
where A' = A^T and pre_j[b,n,s] = c_j * x[b,n,s] + d_j (scalars c_j, d_j, e
derived from w_start/b_start/w_mlp/b_mlp on the host; e folded into pre0).

Quantization: adj is stored fp8 (e3m4) scaled by 16. pre_j planes are
pre-scaled by 16^(3-j) on the host and the final output is divided by 16^3
after the gather.

Device kernel (per core; data-parallel over batch, 4 b per core):
  - adj[b] packed as 84 "quads": 4 A-matrices per 128x128 stationary tile
    (2x2 blocks of 64x64), fp8 e3m4.  Quad block (pb,cb) holds
    A_{s=4q+sigma(pb,cb)}, sigma = [[1,0],[2,3]][pb][cb]; moving col 4q+j
    carries chain s=4q+j in its input half (j0,j1 top; j2,j3 bottom),
    output lands at the class's output half (j1 top/top, j3 bot/bot direct;
    j0 top->bot, j2 bot->top crossed).
  - 3 passes of one matmul per quad. Pass transitions:
      * direct classes (j1, j3): one DVE add psum+pre -> V  per half-class.
      * crossed classes (j0, j2): DVE stages psum+pre pair-interleaved into
        TA, a PE matmul with the 64<->64 partition-swap matrix moves the
        halves, two Scalar copies land them in V.
    During the DMA-delivery-bound early steps (and for the tail batch) the
    shift+copies run inline right after their stage (PE has idle slots);
    in steady state they run at the START of the next step so their inputs
    are a full step old and the PE FIFO never blocks on them.
  - All tiles are per-batch (no parity rotation): adj, host-shipped pre
    planes (pre0..pre2 mirrored + v3 start vectors, [128, 4S] fp16 per
    batch), V1/V2 chain tiles, TA staging, TF output staging.
  - All data DMAs ride the two hardware DGE queues (SP: adj + sh16 +
    outputs; Activation: pre/v3 planes).  gpsimd only does V memsets.
  - PSUM: banks 0-6 = 7 rotating pass-psum slots; bank 7 = 3 rotating
    shift-output slots [128, 168].
  - Final pass lands psum+pre0 in TF[b] (crossed pair block / direct pair
    block); host reassembles column order (free) after the gather.
"""

import sys

sys.path.insert(0, "/opt/trn_rl_repo")

import ml_dtypes
import numpy as np

import concourse.bass as bass
import concourse.mybir as mybir
import concourse.tile as tile
from concourse import bacc
from concourse.bass_utils import run_bass_kernel_spmd

B, N, S, D = 32, 64, 336, 32
MP_LAYERS = 3
PROPBETA = 0.05
NCORES = 8
BL = B // NCORES          # batches per core
Q = S // 4                # quads per batch (84)
H = Q // 2                # half split (42 quads)

ADJ_DT = mybir.dt.float8e3    # e3m4
ADJ_NP = ml_dtypes.float8_e3m4
ADJ_SCALE = 16.0
OUT_DESCALE = float(ADJ_SCALE ** MP_LAYERS)
V_DT = mybir.dt.float16       # chain-vector / pre / out dtype
V_NP = np.float16

f32 = mybir.dt.float32


def _coefficients(w_start, b_start, w_mlp, b_mlp):
    """Collapse the feature dim: out = sum_j A'^j (c_j x + d_j 1) + e (j=0..K)."""
    K = MP_LAYERS
    beta, sb = PROPBETA, 1.0 - PROPBETA
    ws = w_start[0].astype(np.float64)
    bs = b_start.astype(np.float64)
    w = [w_mlp[k * D:(k + 1) * D, 0].astype(np.float64) for k in range(K + 1)]

    u = {(0, 0): ws}
    v = {(0, 0): bs}
    for k in range(K):
        nu = {(k + 1, 0): beta * np.ones(D)}
        nv = {(k + 1, 0): np.zeros(D)}
        for j in range(k + 1):
            nu[(k + 1, j + 1)] = sb * u[(k, j)]
            nv[(k + 1, j + 1)] = sb * v[(k, j)]
        u.update(nu)
        v.update(nv)

    c = np.zeros(K + 1)
    d = np.zeros(K + 1)
    for k in range(K + 1):
        for j in range(k + 1):
            c[j] += float(u[(k, j)] @ w[k])
            d[j] += float(v[(k, j)] @ w[k])
    e = d[0] + float(b_mlp[0])
    return c, d, e


def _shift_matrix():
    sh = np.zeros((128, 128), dtype=np.float32)
    idx = np.arange(64)
    sh[idx, idx + 64] = 1.0
    sh[idx + 64, idx] = 1.0
    return sh


def _qview(ap):
    """[P, S] -> [P, q, f] with f in 0..3 (col = 4q+f)."""
    return ap.rearrange("p (q f) -> p q f", f=4)


def build_nc():
    nc = bacc.Bacc("TRN2", target_bir_lowering=False, debug=False)

    adj_l = nc.dram_tensor("adj", [BL, 128, Q * 128], ADJ_DT,
                           kind="ExternalInput")
    pre_l = nc.dram_tensor("prev3", [BL, 128, 4 * S], V_DT,
                           kind="ExternalInput")
    shift16 = nc.dram_tensor("shift16", [128, 128], V_DT, kind="ExternalInput")
    out_l = nc.dram_tensor("out", [BL, 128, S], V_DT, kind="ExternalOutput")

    with tile.TileContext(nc) as tc:
        with (
            tc.tile_pool(name="singles", bufs=1) as singles,
            tc.tile_pool(name="psb_pool", bufs=1, space=bass.MemorySpace.PSUM)
            as psb_pool,
        ):
            sh16 = singles.tile([128, 128], V_DT, tag="sh16", name="sh16")
            pre_t = [singles.tile([128, 4 * S], V_DT, tag=f"pre{b}",
                                  name=f"pre{b}") for b in range(BL)]
            V = {}
            for b in range(BL):
                for k in (1, 2):
                    V[b, k] = singles.tile([128, S], V_DT,
                                           tag=f"v{b}{k}", name=f"v{b}{k}")
            TA = [singles.tile([128, 2 * Q], V_DT, tag=f"ta{b}", name=f"ta{b}")
                  for b in range(BL)]
            TF = [singles.tile([128, S], V_DT, tag=f"tf{b}", name=f"tf{b}")
                  for b in range(BL)]
            adj_t = [singles.tile([128, Q * 128], ADJ_DT,
                                  tag=f"adj{b}", name=f"adj{b}")
                     for b in range(BL)]

            # ---- all input DMAs up front, hardware DGE queues only ----
            for q0, q1 in ((0, 12), (12, 36), (36, 84)):
                nc.sync.dma_start(out=adj_t[0][:, q0 * 128:q1 * 128],
                                  in_=adj_l[0][:, q0 * 128:q1 * 128])
            nc.sync.dma_start(out=adj_t[1][:, :H * 128],
                              in_=adj_l[1][:, :H * 128])
            nc.sync.dma_start(sh16[:], shift16[:])
            nc.sync.dma_start(out=adj_t[1][:, H * 128:],
                              in_=adj_l[1][:, H * 128:])
            for b in (2, 3):
                nc.sync.dma_start(out=adj_t[b][:, :H * 128],
                                  in_=adj_l[b][:, :H * 128])
                nc.sync.dma_start(out=adj_t[b][:, H * 128:],
                                  in_=adj_l[b][:, H * 128:])

            nc.scalar.dma_start(out=pre_t[0][:, 3 * S:],
                                in_=pre_l[0][:, 3 * S:])
            nc.scalar.dma_start(out=pre_t[0][:, :3 * S],
                                in_=pre_l[0][:, :3 * S])
            for b in (1, 2, 3):
                nc.scalar.dma_start(out=pre_t[b][:], in_=pre_l[b][:])

            for b in range(BL):
                for k in (1, 2):
                    nc.gpsimd.memset(V[b, k][:], 0.0)

            def vsrc(b, k):
                if k == 3:
                    return pre_t[b][:, 3 * S:]
                return V[b, k][:]

            def pre_plane(b, j):
                return pre_t[b][:, j * S:(j + 1) * S]

            psb = [psb_pool.tile([128, 512], f32, tag=f"psb{i}",
                                 name=f"psb{i}") for i in range(8)]
            psums = {}
            shps = {}
            _ctr = {"ps": 0, "sh": 0}

            def ps_of(b, k):
                key = (b, k)
                if key not in psums:
                    s = _ctr["ps"] % 7
                    _ctr["ps"] += 1
                    psums[key] = psb[s][:, 0:336]
                return psums[key]

            def sh_of(b, k):
                key = (b, k)
                if key not in shps:
                    s = _ctr["sh"] % 3
                    _ctr["sh"] += 1
                    shps[key] = psb[7][:, s * 168:(s + 1) * 168]
                return shps[key]

            def p_slice(b, k, q0, q1):
                ps = ps_of(b, k)
                src = vsrc(b, k)
                for q in range(q0, q1):
                    nc.tensor.matmul(
                        ps[:, 4 * q:4 * q + 4],
                        adj_t[b][:, 128 * q:128 * (q + 1)],
                        src[:, 4 * q:4 * q + 4],
                        start=True, stop=True,
                    )

            def stage_a(b, k, q0, q1):
                p = _qview(ps_of(b, k))
                pr = _qview(pre_plane(b, k - 1))
                qs = slice(q0, q1)
                blk = TA[b][:, 2 * q0:2 * q1].rearrange("p (q c) -> p q c",
                                                        c=2)
                nc.vector.tensor_add(blk[:, :, :], p[:, qs, 0:3:2],
                                     pr[:, qs, 0:3:2])

            def b_direct(b, k, q0, q1):
                p = _qview(ps_of(b, k))
                pr = _qview(pre_plane(b, k - 1))
                vn = _qview(V[b, k - 1][:])
                qs = slice(q0, q1)
                nc.vector.tensor_add(vn[0:64, qs, 1], p[0:64, qs, 1],
                                     pr[0:64, qs, 1])
                nc.vector.tensor_add(vn[64:128, qs, 3], p[64:128, qs, 3],
                                     pr[64:128, qs, 3])

            def shift(b, k, q0, q1):
                sv = sh_of(b, k)
                nc.tensor.matmul(sv[:, 2 * q0:2 * q1], sh16[:],
                                 TA[b][:, 2 * q0:2 * q1],
                                 start=True, stop=True,
                                 skip_group_check=True)

            def copies(b, k, q0, q1):
                vn = _qview(V[b, k - 1][:])
                sv = sh_of(b, k).rearrange("p (q t) -> p q t", t=2)
                qs = slice(q0, q1)
                nc.scalar.copy(vn[0:64, qs, 0], sv[0:64, qs, 0])
                nc.scalar.copy(vn[64:128, qs, 2], sv[64:128, qs, 1])

            def f_stage(b, q0, q1):
                p1 = _qview(ps_of(b, 1))
                pr = _qview(pre_plane(b, 0))
                qs = slice(q0, q1)
                blka = TF[b][:, 2 * q0:2 * q1].rearrange("p (q c) -> p q c",
                                                         c=2)
                blkb = TF[b][:, 168 + 2 * q0:168 + 2 * q1].rearrange(
                    "p (q c) -> p q c", c=2)
                nc.vector.tensor_add(blka[:, :, :], p1[:, qs, 0:4:3],
                                     pr[:, qs, 0:4:3])
                nc.vector.tensor_add(blkb[:, :, :], p1[:, qs, 1:3],
                                     pr[:, qs, 1:3])

            def trans_inline(b, k, q0, q1):
                stage_a(b, k, q0, q1)
                b_direct(b, k, q0, q1)
                shift(b, k, q0, q1)
                copies(b, k, q0, q1)

            def trans_stage(b, k, q0, q1):
                stage_a(b, k, q0, q1)
                b_direct(b, k, q0, q1)

            # ---- 3-deep skewed software pipeline, half-granular -----------
            # Crossed-class transitions straddle step boundaries: pass-k
            # psums are staged (DVE) at the end of the step that produced
            # them, the shift matmuls + copies run AT THE START of the next
            # step (inputs a full step old -> no PE stall), before the
            # dependent pass-(k-1) matmuls of the same batch run mid-step.
            # Exception: the tail batch's k2 transition is inline at the
            # end of s4 (nothing else to overlap with in s5 anyway).
            for s in range(BL + 2):
                c = s if s < BL else None                  # pass-3 batch
                b = s - 1 if 0 <= s - 1 < BL else None     # pass-2 batch
                a = s - 2 if 0 <= s - 2 < BL else None     # pass-1 batch

                if a is not None and a < BL - 1:
                    shift(a, 2, 0, H)
                    copies(a, 2, 0, H)
                if b is not None:
                    shift(b, 3, 0, H)
                    copies(b, 3, 0, H)
                if a is not None and a < BL - 1:
                    shift(a, 2, H, Q)
                    copies(a, 2, H, Q)
                if b is not None:
                    shift(b, 3, H, Q)
                    copies(b, 3, H, Q)

                if a is not None:
                    p_slice(a, 1, 0, H)
                    f_stage(a, 0, H)
                if b is not None:
                    p_slice(b, 2, 0, H)
                    trans_stage(b, 2, 0, H)
                    if b == BL - 1:
                        shift(b, 2, 0, H)
                        copies(b, 2, 0, H)
                if a is not None:
                    p_slice(a, 1, H, Q)
                    f_stage(a, H, Q)
                    nc.sync.dma_start(out=out_l[a], in_=TF[a][:])
                if b is not None:
                    p_slice(b, 2, H, Q)
                    trans_stage(b, 2, H, Q)
                    if b == BL - 1:
                        shift(b, 2, H, Q)
                        copies(b, 2, H, Q)
                if c is not None:
                    if c == 0:
                        # paced to the adj chunk arrivals (12, 24, 48)
                        p_slice(0, 3, 0, 12)
                        p_slice(0, 3, 12, 36)
                        p_slice(0, 3, 36, H)
                    else:
                        p_slice(c, 3, 0, H)
                    trans_stage(c, 3, 0, H)
                    p_slice(c, 3, H, Q)
                    trans_stage(c, 3, H, Q)

    nc.finalize()
    return nc


_NC_CACHE = None


def _get_nc():
    global _NC_CACHE
    if _NC_CACHE is None:
        _NC_CACHE = build_nc()
    return _NC_CACHE


def _pack_adj(adj):
    """[B, S, N, N] f32 -> [B, 128, Q*128] fp8 (x16) quad layout."""
    sigma = np.array([[1, 0], [2, 3]])  # [pb][cb]
    s_idx = 4 * np.arange(Q)[:, None, None] + sigma[None, :, :]
    a = adj[:, s_idx]                      # [B, Q, 2pb, 2cb, n, m]
    a = a.transpose(0, 2, 4, 1, 3, 5)      # [B, pb, n, Q, cb, m]
    return np.ascontiguousarray(
        (a.reshape(B, 128, Q * 128) * ADJ_SCALE).astype(ADJ_NP))


def _prepare_in_maps(x, adj, w_start, b_start, w_mlp, b_mlp):
    c, d, e = _coefficients(np.asarray(w_start), np.asarray(b_start),
                            np.asarray(w_mlp), np.asarray(b_mlp))
    x = np.asarray(x, dtype=np.float32)
    adj = _pack_adj(np.asarray(adj, dtype=np.float32))
    prev3 = np.zeros((B, 128, 4 * S), dtype=V_NP)
    for j in range(MP_LAYERS):
        sc = ADJ_SCALE ** (MP_LAYERS - j)
        plane = (c[j] * sc * x + (e if j == 0 else d[j]) * sc).astype(V_NP)
        prev3[:, 0:64, j * S:(j + 1) * S] = plane
        prev3[:, 64:128, j * S:(j + 1) * S] = plane
    pre3 = (c[MP_LAYERS] * x + d[MP_LAYERS]).astype(V_NP)  # [B, N, S]
    p3q = pre3.reshape(B, N, Q, 4)
    v3 = np.zeros((B, 128, Q, 4), dtype=V_NP)
    v3[:, 0:64, :, 0:2] = p3q[:, :, :, 0:2]
    v3[:, 64:128, :, 2:4] = p3q[:, :, :, 2:4]
    prev3[:, :, 3 * S:] = v3.reshape(B, 128, S)
    sh = _shift_matrix().astype(V_NP)
    in_maps = []
    for i in range(NCORES):
        sl = slice(i * BL, (i + 1) * BL)
        in_maps.append({
            "adj": np.ascontiguousarray(adj[sl]),
            "prev3": np.ascontiguousarray(prev3[sl]),
            "shift16": sh,
        })
    return in_maps


def run_spmd(inputs, trace=False, **kw):
    in_maps = _prepare_in_maps(**inputs)
    res = run_bass_kernel_spmd(_get_nc(), in_maps,
                               core_ids=list(range(NCORES)), trace=trace, **kw)
    tfd = np.concatenate([r["out"] for r in res.results],
                         axis=0).astype(np.float32)
    # staged blocks: A = (f0, f3) pair-interleaved, useful at partitions
    # 64:128; B = (f1, f2), useful at 0:64 (garbage halves ignored)
    a = tfd[:, 64:128, 0:2 * Q].reshape(B, N, Q, 2)
    bb = tfd[:, 0:64, 2 * Q:4 * Q].reshape(B, N, Q, 2)
    out = np.empty((B, N, Q, 4), dtype=np.float32)
    out[..., 0] = a[..., 0]
    out[..., 3] = a[..., 1]
    out[..., 1] = bb[..., 0]
    out[..., 2] = bb[..., 1]
    return out.reshape(B, N, S) / OUT_DESCALE, res


def kernel(**inputs):
    out, _ = run_spmd(inputs)
    return out.astype(np.float32)


if __name__ == "__main__":
    # quick smoke test against a numpy oracle
    rng = np.random.default_rng(0)
    x = rng.standard_normal((B, N, S), dtype=np.float32)
    adj = rng.random((B, S, N, N), dtype=np.float32)
    adj /= adj.sum(-1, keepdims=True)
    w_start = rng.standard_normal((1, D)).astype(np.float32)
    b_start = (rng.standard_normal(D) * 0.01).astype(np.float32)
    w_mlp = (rng.standard_normal(((MP_LAYERS + 1) * D, 1)) /
             np.sqrt((MP_LAYERS + 1) * D)).astype(np.float32)
    b_mlp = (rng.standard_normal(1) * 0.01).astype(np.float32)

    got = kernel(x=x, adj=adj, w_start=w_start, b_start=b_start,
                 w_mlp=w_mlp, b_mlp=b_mlp)

    h = x[..., None] * w_start[0] + b_start
    outs = [h]
    a = np.transpose(adj, (0, 2, 3, 1))
    for _ in range(MP_LAYERS):
        conv = np.einsum('bnsc,bnms->bmsc', h, a, optimize=True)
        h = PROPBETA * x[..., None] + (1 - PROPBETA) * conv
        outs.append(h)
    hc = np.concatenate(outs, axis=-1)
    want = (hc @ w_mlp)[..., 0] + b_mlp[0]

    aerr = np.abs(got - want)
    print("max abs err:", aerr.max(),
          "normalized:", aerr.max() / np.abs(want).max())
